# revision 1
# baseline (speedup 1.0000x reference)
"""CharacterAwareEncoder kernel for Trainium2 (8 NeuronCores, data-parallel).

reference:
    word_embeds  = word_emb_table[word_ids]                  # [B, S, 412] gather
    char_features = sin(freqs * word_ids), 0 where id == 0   # [B, S, 100]
    out = concat([word_embeds, char_features], -1)           # [B, S, 512]

Per core (4096 tokens):
  - Gather: table host-padded to a 448-col (1792 B, 256-byte-multiple)
    row stride; 8+8 Pool dma_gather instructions of 512 tokens each, split
    by column group (0:256 via the bass API, 256:412 via a raw-constructed
    gather whose elem_size skips the pad — only the stride needs 256-byte
    granularity).  The first group's store streams out while the second
    group is still gathering; token slot g lands at SBUF [g%128, g//128].
  - Sin (all ops walrus-valid): with host-prescaled freqs2 = h/(2000*pi),
    u = tok * freqs2 counts the angle in turns; kf = (u+C)-C (C = 1.5*2^23
    round-to-nearest magic) in one fused tensor_scalar; d = u - kf is the
    centered fractional turn in [-1/2, 1/2]; ACT Sin(scale=2pi*(1-eps))
    evaluates sin(2pi*d) = sin(x) with the operand strictly inside the
    Sin table domain [-pi, pi].  sin(0)=0 reproduces the id==0 masking.
  - Stores: token slots are host-permuted so SBUF flat order (p-major)
    equals DRAM row order; three strided DRAM-row APs (cols 0:256,
    256:412, 412:512) stream out on SP at the per-row modeled cost.
"""

import numpy as np

import concourse.bacc as bacc
import concourse.bass as bass
import concourse.mybir as mybir
import concourse.tile as tile
from concourse.bass_utils import run_bass_kernel_spmd

B, S = 16, 2048
V, D, H = 32000, 412, 100
OUT_D = 512
N_CORES = 8
P = 128
T_CORE = B * S // N_CORES          # 4096 tokens per core
N_TILES = T_CORE // P              # 32
PAD_D = 448
N_GATHERS = 8
TOK_G = T_CORE // N_GATHERS        # 512 tokens per gather instruction
# (engine, n_tiles) chain plan; DVE entries run first, Pool entries are
# emitted after the gathers on the Pool queue.
DVE_PLAN = [8, 8, 8, 8]
POOL_PLAN = []
TILES_G = 8                        # max tiles per chain group (broadcast width)
W = TILES_G * H

_f32 = mybir.dt.float32
_i64 = mybir.dt.int64
_i32 = mybir.dt.int32
_i16 = mybir.dt.int16

C_MAGIC = float(np.float32(3 << 22))          # 1.5 * 2^23
SIN_SCALE = float(np.float32(2.0 * np.pi) * np.float32(1.0 - 1.2e-7))
# number of trailing chain groups computed on Pool (after the gathers)
POOL_GROUPS = 1

_NC = {}


def _dma_gather_raw(gp, out_ap, in_ap, idxs_ap, num_idxs, elem_size, elem_step):
    """bass.dma_gather minus its elem_size_bytes%256 assert (that rule is a
    transpose-mode restriction; only the row stride is encoded in 256-byte
    units).  Non-transpose, HBM->SBUF, gen_mode 0 only.  Device-validated."""
    stride_bytes = elem_step * 4
    assert stride_bytes % 256 == 0
    assert in_ap.ap[0][0] == elem_step and in_ap.ap[-1][1] == elem_size
    return gp.add_instruction(
        mybir.InstDMAGatherAnt(
            name=gp.bass.get_next_instruction_name(),
            ins=[*gp.lower_ap_dma(in_ap, for_custom_bir_dma=True),
                 gp.lower_ap(idxs_ap),
                 gp.lower_val_access(gp.to_reg(num_idxs))],
            outs=[gp.lower_ap(out_ap)],
            transpose=False,
            num_idxs=num_idxs,
            elem_size=elem_size,
            stride_bytes_256=stride_bytes // 256,
            gen_mode=0,
            single_packet=True,
            queue_num=0,
            sbuf_tokens_per_rank=0,
            sbuf_free_dim_per_rank=0,
            sbuf_free_dim_pad_per_rank=0,
            sbuf_byte_offset=0,
        )
    )


def _build_nc():
    nc = bacc.Bacc("TRN2", target_bir_lowering=False)
    # packed input: [0:256] int16 idx, [256:...] f32 consts (viewed)
    NC16 = T_CORE // 16 + 2 * (H + N_TILES)
    packed_t = nc.dram_tensor("packed", [P, NC16], _i16, kind="ExternalInput")
    table_t = nc.dram_tensor("table", [V, PAD_D], _f32, kind="ExternalInput")
    out_t = nc.dram_tensor("out", [T_CORE, OUT_D], _f32, kind="ExternalOutput")

    with tile.TileContext(nc) as tc:
        with tc.tile_pool(name="main", bufs=1) as pool:
            packed_sb = pool.tile([P, NC16], _i16)
            gbufA = pool.tile([P, N_TILES, 256], _f32)
            gbufB = pool.tile([P, N_TILES, D - 256], _f32)
            x = pool.tile([P, N_TILES, H], _f32)
            kf = pool.tile([P, N_TILES, H], _f32)
            d = pool.tile([P, N_TILES, H], _f32)
            s = pool.tile([P, N_TILES, H], _f32)
            warm = pool.tile([P, 1], _f32)

            nc.sync.dma_start(out=packed_sb[:], in_=packed_t[:])
            idx_sb = packed_sb[:, 0 : T_CORE // 16]
            cview = packed_sb[:].bitcast(_f32)  # [P, NC16//2]
            c0 = T_CORE // 32
            freqs2 = cview[:, c0 : c0 + H]      # h/(2000*pi)
            tokf = cview[:, c0 + H : c0 + H + N_TILES]
            def freqs2_b(n):
                return freqs2.rearrange("p (g h) -> p g h", g=1).broadcast_to(
                    [P, n, H])

            # ACT Sin table warm-up (freqs2[:,0] == 0.0 -> sin(0)).
            nc.scalar.activation(
                out=warm[:], in_=freqs2[:, 0:1],
                func=mybir.ActivationFunctionType.Sin,
            )

            tg = TOK_G // P
            for c in range(N_GATHERS):
                nc.gpsimd.dma_gather(
                    gbufA[:, c * tg : (c + 1) * tg, :], table_t[:, 0:256],
                    packed_sb[:, c * (TOK_G // 16) : (c + 1) * (TOK_G // 16)],
                    TOK_G, TOK_G, 256, elem_step=PAD_D,
                )
            for c in range(N_GATHERS):
                _dma_gather_raw(
                    nc.gpsimd, gbufB[:, c * tg : (c + 1) * tg, :],
                    table_t[:, 256:D],
                    packed_sb[:, c * (TOK_G // 16) : (c + 1) * (TOK_G // 16)],
                    TOK_G, D - 256, PAD_D,
                )

            def chain(eng, t0, n):
                sl = slice(t0, t0 + n)
                eng.tensor_tensor(
                    out=x[:, sl, :],
                    in0=tokf[:, sl].to_broadcast([P, n, H]),
                    in1=freqs2_b(n),
                    op=mybir.AluOpType.mult,
                )
                eng.tensor_scalar(
                    out=kf[:, sl, :], in0=x[:, sl, :],
                    scalar1=C_MAGIC, scalar2=C_MAGIC,
                    op0=mybir.AluOpType.add, op1=mybir.AluOpType.subtract,
                )
                eng.tensor_tensor(
                    out=d[:, sl, :], in0=x[:, sl, :], in1=kf[:, sl, :],
                    op=mybir.AluOpType.subtract,
                )
                nc.scalar.activation(
                    out=s[:, sl, :], in_=d[:, sl, :],
                    func=mybir.ActivationFunctionType.Sin,
                    scale=SIN_SCALE,
                )

            t0 = 0
            for n in DVE_PLAN:
                chain(nc.vector, t0, n)
                t0 += n
            for n in POOL_PLAN:
                chain(nc.gpsimd, t0, n)
                t0 += n
            assert t0 == N_TILES

            nc.sync.dma_start(out=out_t[:, 0:256], in_=gbufA[:])
            nc.sync.dma_start(out=out_t[:, D:OUT_D], in_=s[:])
            nc.sync.dma_start(out=out_t[:, 256:D], in_=gbufB[:])
    nc.compile()
    return nc


def _get_nc(mode=None):
    if "nc" not in _NC:
        _NC["nc"] = _build_nc()
    return _NC["nc"]


def make_in_maps(word_ids, word_emb_table, mode=None):
    ids = np.ascontiguousarray(np.asarray(word_ids)).astype(np.int32).reshape(-1)
    table = np.asarray(word_emb_table, dtype=np.float32)
    padded = np.zeros((V, PAD_D), np.float32)
    padded[:, 0:D] = table


    freqs2_row = (np.arange(H, dtype=np.float64) / (2000.0 * np.pi)).astype(
        np.float32)  # [H]

    # slot permutation: gather slot g holds the token that must land in DRAM
    # row (g%128)*N_TILES + g//128, i.e. slot order is p-major flat order.
    slot_to_row = (np.arange(T_CORE) % P) * N_TILES + np.arange(T_CORE) // P

    in_maps = []
    for c in range(N_CORES):
        shard = ids[c * T_CORE : (c + 1) * T_CORE]
        slot_ids = shard[slot_to_row].astype(np.int16)       # [T_CORE]
        idx16 = slot_ids.reshape(T_CORE // 16, 16).T         # [16, T/16]
        idx16 = np.ascontiguousarray(np.tile(idx16, (8, 1)))  # [128, T/16]
        consts = np.empty((P, H + N_TILES), np.float32)
        consts[:, 0:H] = freqs2_row
        consts[:, H : H + N_TILES] = shard.reshape(P, N_TILES).astype(np.float32)
        packed = np.concatenate([idx16, consts.view(np.int16)], axis=1)
        in_maps.append({"packed": np.ascontiguousarray(packed),
                        "table": padded})
    return in_maps


def kernel(word_ids, word_emb_table):
    import jax
    nc = _get_nc()
    in_maps = make_in_maps(word_ids, word_emb_table)
    res = run_bass_kernel_spmd(nc, in_maps, core_ids=list(range(N_CORES)))
    out = np.concatenate([r["out"] for r in res.results], axis=0)
    return out.reshape(B, S, OUT_D)



# revision 23
# speedup vs baseline: 1.0478x; 1.0478x over previous
"""CharacterAwareEncoder kernel for Trainium2 (8 NeuronCores, data-parallel).

reference:
    word_embeds  = word_emb_table[word_ids]                  # [B, S, 412] gather
    char_features = sin(freqs * word_ids), 0 where id == 0   # [B, S, 100]
    out = concat([word_embeds, char_features], -1)           # [B, S, 512]

Per core (4096 tokens):
  - Gather: table host-padded to a 448-col (1792 B) row stride; Pool
    dma_gather instructions fetch 412 f32 (1648 B) per token in one shot via
    a raw-constructed gather (the bass API's elem_size%256 assert is a
    transpose-mode rule; only the row stride needs 256-byte granularity).
    Device limits (probed): 4-byte dtypes only (int64/uint64 crash) and at
    most 512 indices per instruction, so chunks are <=4 tiles.  The
    serialized Pool gather stream (~11 us) is the critical path; the whole
    sin pipeline and all but the last stores hide under it.
  - Layout: gather chunk c owns the contiguous DRAM row range
    [128*t0_c, 128*t1_c); within a chunk, slot j (SBUF partition j%128,
    position t0_c + j//128) holds the token of row 128*t0_c + (j%128)*n_c +
    j//128, so each chunk's SBUF (p, pos) lexicographic order equals its own
    DRAM row order and it can stream out in a cheap 2-dim row-contiguous
    DMA the moment it lands.
  - Tail: a DMA copy has ~2.3 us issue-to-completion latency, so the last
    SCATTER_TILES tiles instead go out via Pool dma_scatter_add (destination
    rows as indices, ~0.35 us/tile) onto rows zero-filled early by a cheap
    broadcast DMA; the scatters' out APs are narrowed to their own row
    ranges so they don't serialize against the earlier chunk stores.
  - Sin (all ops device-validated): with host-prescaled freqs2 = h/(2000*pi),
    x = tok * freqs2 counts the angle in turns (DVE tensor_tensor,
    broadcast); kf = (x+C)-C (C = 1.5*2^23 round-to-nearest magic, exact RNE
    f32 in both CoreSim and HW) in one fused DVE tensor_scalar (2x mode);
    d = x - kf is the centered fractional turn in [-1/2, 1/2]; ACT
    Sin(scale=2pi*(1-eps)) evaluates sin(2pi*d) = sin(freqs*tok) strictly
    inside the Sin table domain [-pi, pi].  tok==0 -> d=0 -> sin(0)=0
    reproduces the id==0 mask.  Sin chunks align to gather-chunk boundaries
    and stream out in row-contiguous DMAs on the ACT queue, all finishing
    under the gather stream.
"""

import numpy as np

import concourse.bacc as bacc
import concourse.bass as bass
import concourse.mybir as mybir
import concourse.tile as tile
from concourse.bass_utils import run_bass_kernel_spmd

B, S = 16, 2048
V, D, H = 32000, 412, 100
OUT_D = 512
N_CORES = 8
P = 128
T_CORE = B * S // N_CORES          # 4096 tokens per core
N_TILES = T_CORE // P              # 32 within-partition positions
PAD_D = 448                        # f32 cols incl pad (1792 B row stride)

# gather chunks in tiles (<=4 per the 512-idx device limit).  The final
# SCATTER_PLAN tiles leave via scatters; everything earlier via DMA copies.
GATHER_PLAN = [4, 4, 4, 4, 4, 4, 4, 2, 1, 1]
SCATTER_PLAN = [2, 1, 1]           # last chunks (must match GATHER_PLAN tail)
SIN_PLAN = [8, 8, 8, 4, 2, 1, 1]   # sin chunks; cum sums on gather boundaries
ACT_TILES = 14                     # trailing tiles whose x is computed on ACT
DVE_TTM_PLAN = [9, 9]              # x = tok*freqs2 chunking on DVE

_f32 = mybir.dt.float32
_i16 = mybir.dt.int16

SIN_SCALE = float(np.float32(2.0 * np.pi) * np.float32(1.0 - 1.2e-7))
C_MAGIC = float(np.float32(3 << 22))          # 1.5 * 2^23

_NC = {}


def _dma_gather_raw(gp, out_ap, in_ap, idxs_ap, num_idxs, elem_size, elem_step,
                    dt_size):
    """bass.dma_gather minus its elem_size_bytes%256 assert (that rule is a
    transpose-mode restriction; only the row stride is encoded in 256-byte
    units).  Non-transpose, HBM->SBUF, gen_mode 0 only.  Device-validated."""
    stride_bytes = elem_step * dt_size
    assert stride_bytes % 256 == 0
    assert in_ap.ap[0][0] == elem_step and in_ap.ap[-1][1] == elem_size
    assert num_idxs <= 512
    return gp.add_instruction(
        mybir.InstDMAGatherAnt(
            name=gp.bass.get_next_instruction_name(),
            ins=[*gp.lower_ap_dma(in_ap, for_custom_bir_dma=True),
                 gp.lower_ap(idxs_ap),
                 gp.lower_val_access(gp.to_reg(num_idxs))],
            outs=[gp.lower_ap(out_ap)],
            transpose=False,
            num_idxs=num_idxs,
            elem_size=elem_size,
            stride_bytes_256=stride_bytes // 256,
            gen_mode=0,
            single_packet=True,
            queue_num=0,
            sbuf_tokens_per_rank=0,
            sbuf_free_dim_per_rank=0,
            sbuf_byte_offset=0,
            sbuf_free_dim_pad_per_rank=0,
        )
    )


def _gather_bounds():
    cum = [0]
    for n in GATHER_PLAN:
        cum.append(cum[-1] + n)
    assert cum[-1] == N_TILES
    return cum


def _build_nc():
    nc = bacc.Bacc("TRN2", target_bir_lowering=False)
    n_scatter = sum(SCATTER_PLAN)
    assert GATHER_PLAN[-len(SCATTER_PLAN):] == SCATTER_PLAN
    sc_t0 = N_TILES - n_scatter
    # packed input: [0:256] i16 gather idx, then i16 scatter row idx per
    # scatter chunk, then f32 consts (viewed)
    NSC = n_scatter * (P // 16)
    NC16 = T_CORE // 16 + NSC + 2 * (H + N_TILES)
    packed_t = nc.dram_tensor("packed", [P, NC16], _i16, kind="ExternalInput")
    table_t = nc.dram_tensor("table", [V, PAD_D], _f32, kind="ExternalInput")
    out_t = nc.dram_tensor("out", [T_CORE, OUT_D], _f32, kind="ExternalOutput")

    with tile.TileContext(nc) as tc:
        with tc.tile_pool(name="main", bufs=1) as pool:
            packed_sb = pool.tile([P, NC16], _i16)
            gbuf = pool.tile([P, N_TILES, D], _f32)
            x = pool.tile([P, N_TILES, H], _f32)
            kf = pool.tile([P, N_TILES, H], _f32)
            d = pool.tile([P, N_TILES, H], _f32)
            s = pool.tile([P, N_TILES, H], _f32)
            warm = pool.tile([P, 1], _f32)
            zsrc = pool.tile([P, D], _f32)

            nc.gpsimd.memset(zsrc[:], 0.0)
            nc.sync.dma_start(out=packed_sb[:], in_=packed_t[:])
            # zero the scatter chunks' rows early; scatters accumulate onto
            # them.
            nc.sync.dma_start(
                out=out_t[sc_t0 * P : N_TILES * P, 0:D],
                in_=zsrc[:].rearrange("p (g c) -> p g c", g=1).broadcast_to(
                    [P, n_scatter, D]),
            )
            cview = packed_sb[:].bitcast(_f32)  # [P, NC16//2]
            c0 = (T_CORE // 16 + NSC) // 2
            freqs2 = cview[:, c0 : c0 + H]      # h/(2000*pi)
            tokf = cview[:, c0 + H : c0 + H + N_TILES]

            # ACT Sin table warm-up (freqs2[:,0] == 0.0 -> sin(0)).
            nc.scalar.activation(
                out=warm[:], in_=freqs2[:, 0:1],
                func=mybir.ActivationFunctionType.Sin,
                scale=SIN_SCALE,
            )

            # --- gathers on Pool; DMA-copy stores on SP for the leading
            #     chunks; narrowed scatters for the trailing ones ---
            t0 = 0
            for n in GATHER_PLAN:
                _dma_gather_raw(
                    nc.gpsimd, gbuf[:, t0 : t0 + n, :], table_t[:, 0:D],
                    packed_sb[:, t0 * 8 : (t0 + n) * 8],
                    n * P, D, PAD_D, 4,
                )
                t0 += n
                if t0 <= sc_t0:
                    nc.sync.dma_start(
                        out=out_t[(t0 - n) * P : t0 * P, 0:D],
                        in_=gbuf[:, t0 - n : t0, :])
            assert t0 == N_TILES
            sid = T_CORE // 16
            t0 = sc_t0
            for n in SCATTER_PLAN:
                nc.gpsimd.dma_scatter_add(
                    out_t[t0 * P : (t0 + n) * P, 0:D],
                    gbuf[:, t0 : t0 + n, :],
                    packed_sb[:, sid : sid + n * (P // 16)],
                    n * P, n * P, D, elem_step=OUT_D,
                )
                sid += n * (P // 16)
                t0 += n

            # --- sin chain: DVE x/round/subtract, ACT Sin, ACT-queue
            #     row-contiguous stores per gather chunk ---
            def freqs2_b(n):
                return freqs2.rearrange("p (g h) -> p g h", g=1).broadcast_to(
                    [P, n, H])

            slices = []
            t0 = 0
            for n in SIN_PLAN:
                slices.append(slice(t0, t0 + n))
                t0 += n
            assert t0 == N_TILES
            gcum = _gather_bounds()
            for sl in slices:
                assert sl.start in gcum and sl.stop in gcum, (
                    "sin chunks must align to gather-chunk row groups")

            t0 = 0
            for n in DVE_TTM_PLAN:
                sl = slice(t0, t0 + n)
                nc.vector.tensor_tensor(
                    out=x[:, sl, :],
                    in0=tokf[:, sl].to_broadcast([P, n, H]),
                    in1=freqs2_b(n),
                    op=mybir.AluOpType.mult,
                )
                t0 += n
            assert t0 == N_TILES - ACT_TILES
            for t in range(N_TILES - ACT_TILES, N_TILES):
                nc.scalar.activation(
                    out=x[:, t, :], in_=freqs2[:, :],
                    func=mybir.ActivationFunctionType.Identity,
                    scale=tokf[:, t : t + 1],
                )

            for sl in slices:
                nc.vector.tensor_scalar(
                    out=kf[:, sl, :], in0=x[:, sl, :],
                    scalar1=C_MAGIC, scalar2=C_MAGIC,
                    op0=mybir.AluOpType.add, op1=mybir.AluOpType.subtract,
                )
                nc.vector.tensor_tensor(
                    out=d[:, sl, :], in0=x[:, sl, :], in1=kf[:, sl, :],
                    op=mybir.AluOpType.subtract,
                )
                nc.scalar.activation(
                    out=s[:, sl, :], in_=d[:, sl, :],
                    func=mybir.ActivationFunctionType.Sin,
                    scale=SIN_SCALE,
                )
                # stores split per gather chunk: each chunk's SBUF (p, pos)
                # order matches its own contiguous row range only.
                # Alternate queues so neither SP nor ACT piles up.
                for g0, g1 in zip(gcum, gcum[1:]):
                    if sl.start <= g0 and g1 <= sl.stop:
                        eng = nc.scalar if (g0 // 4) % 2 else nc.sync
                        eng.dma_start(
                            out=out_t[g0 * P : g1 * P, D:OUT_D],
                            in_=s[:, g0:g1, :])
    nc.compile()
    return nc


def _get_nc(mode=None):
    if "nc" not in _NC:
        _NC["nc"] = _build_nc()
    return _NC["nc"]


def _row_of():
    """row_of[p, pos]: DRAM row of the token at SBUF (p, pos) under the
    chunk-grouped layout."""
    rows = np.empty((P, N_TILES), np.int64)
    t0 = 0
    for n in GATHER_PLAN:
        p = np.arange(P)[:, None]
        k = np.arange(n)[None, :]
        rows[:, t0 : t0 + n] = t0 * P + p * n + k
        t0 += n
    return rows


def make_in_maps(word_ids, word_emb_table, mode=None):
    ids = np.ascontiguousarray(np.asarray(word_ids)).astype(np.int32).reshape(-1)
    table = np.asarray(word_emb_table, dtype=np.float32)
    padded = np.zeros((V, PAD_D), np.float32)
    padded[:, 0:D] = table

    freqs2_row = (np.arange(H, dtype=np.float64) / (2000.0 * np.pi)).astype(
        np.float32)  # [H]

    row_of = _row_of()                       # [P, N_TILES]
    # gather slot order: chunk-local slot j -> (p=j%128, pos=t0+j//128);
    # global slot g = 128*t0 + j holds the token of row row_of[p, pos].
    slot_rows = np.empty(T_CORE, np.int64)
    t0 = 0
    for n in GATHER_PLAN:
        j = np.arange(n * P)
        slot_rows[t0 * P + j] = t0 * P + (j % P) * n + j // P
        t0 += n

    def wrap16(v16):
        w = v16.reshape(len(v16) // 16, 16).T
        return np.ascontiguousarray(np.tile(w, (8, 1)))

    # scatter row indices (local to each scatter chunk's narrowed out AP)
    sc_parts = []
    for n in SCATTER_PLAN:
        j = np.arange(n * P)
        sc_parts.append(((j % P) * n + j // P).astype(np.int16))
    scatter_rows = np.concatenate(sc_parts)

    in_maps = []
    for c in range(N_CORES):
        shard = ids[c * T_CORE : (c + 1) * T_CORE]
        idx16 = wrap16(shard[slot_rows].astype(np.int16))     # [128, T/16]
        sidx16 = wrap16(scatter_rows)
        consts = np.empty((P, H + N_TILES), np.float32)
        consts[:, 0:H] = freqs2_row
        consts[:, H : H + N_TILES] = shard[row_of].astype(np.float32)
        packed = np.concatenate([idx16, sidx16, consts.view(np.int16)],
                                axis=1)
        in_maps.append({"packed": np.ascontiguousarray(packed),
                        "table": padded})
    return in_maps


def kernel(word_ids, word_emb_table):
    nc = _get_nc()
    in_maps = make_in_maps(word_ids, word_emb_table)
    res = run_bass_kernel_spmd(nc, in_maps, core_ids=list(range(N_CORES)))
    out = np.concatenate([r["out"] for r in res.results], axis=0)
    return out.reshape(B, S, OUT_D)
